# revision 10
# baseline (speedup 1.0000x reference)
"""Trainium2 Bass kernel for nn_MultiHeadAttention (B=2, T=2048, M=2048, H=16, D=128).

Sharding: 8 cores = batch(2) x head-groups(4).  Each core computes, for its
batch b and 4 heads: q/k/v projections, qk-RMSNorm, RoPE, causal attention,
and its partial contribution o @ wo to the output.  Host sums the 4 head-group
partials per batch.

All matmuls run in bf16 (fp32 accumulation in PSUM); norm/rope/softmax in fp32.
"""
import sys

BASS_PATH = "/opt/trn_rl_repo"
if BASS_PATH not in sys.path:
    sys.path.insert(0, BASS_PATH)

import numpy as np
from contextlib import ExitStack

import concourse.bass as bass
import concourse.tile as tile
from concourse import mybir
from concourse.bass_utils import run_bass_kernel_spmd
from concourse.vector_clock import ScopedClock
from concourse.masks import make_identity

FP32 = mybir.dt.float32
BF16 = mybir.dt.bfloat16

B, T, M, D = 2, 2048, 2048, 128
H = M // D                      # 16 heads total
HPC = 4                         # heads per core
N_CORES = 8
QK_SCALE = 1.0 / D
NORM_EPS = 1e-6
ROTARY_BASE = 10000.0
NEG_INF = -1e30


def _max_waits(inst):
    # The pinned walrus accepts a single sync-wait command per instruction.
    return 1


class SplitDrainTileContext(tile.TileContext):
    """TileContext that splits excess sem waits across nofuse NOPs.

    The pinned walrus rejects too many sync-wait commands on an instruction
    ("Too many sync wait commands"); distribute the excess one-per-NOP on
    the same engine ahead of the instruction.
    """

    def _commit_and_lower(self, inst, original_block, old_bb_map, bb_to_exit_bb):
        si = getattr(inst, "sync_info", None)
        eng = getattr(inst, "engine", None)
        cap = _max_waits(inst)
        if (si is not None and si.on_wait and len(si.on_wait) > cap
                and eng is not None and eng != mybir.EngineType.Unassigned):
            waits = list(si.on_wait)
            excess, keep = waits[:-cap], waits[-cap:]
            inst.sync_info = mybir.SyncInfo(
                on_wait=keep, on_update=list(si.on_update or []))
            for w in excess:
                nop = mybir.InstNoOp(
                    name=self.nc.get_next_instruction_name(),
                    engine=eng,
                    bass_nofuse=True,
                    sync_info=mybir.SyncInfo(on_wait=[w], on_update=[]),
                )
                super()._commit_and_lower(nop, original_block, old_bb_map,
                                          bb_to_exit_bb)
        return super()._commit_and_lower(inst, original_block, old_bb_map,
                                         bb_to_exit_bb)

    def _drain_and_barrier(self, tick_clock, wait_clock):
        probe = self.nc.sync.nop(nofuse=True)
        wait_clock.add_sem_waits(probe.ins, ScopedClock({None: tick_clock.global_clock}))
        si = probe.ins.sync_info
        waits = list(si.on_wait) if si and si.on_wait else []
        updates = list(si.on_update) if si and si.on_update else []
        if len(waits) > 1:
            probe.ins.sync_info = mybir.SyncInfo(on_wait=waits[:1], on_update=updates)
            for w in waits[1:]:
                nop = self.nc.sync.nop(nofuse=True)
                nop.ins.sync_info = mybir.SyncInfo(on_wait=[w], on_update=[])
        self.nc.sync.drain()
        self.nc.all_engine_barrier()
        popped = self.nc._tile_sem_poison_stack.pop()
        assert popped is self._sem_poison
        self.nc.clear_and_free_semaphores(list(self.sems.allocated().values()))
        self.nc.all_engine_barrier()


def build_nc(t_len=T):
    """Emit the per-core SPMD program.  t_len parameterized for small-scale sim."""
    NT = t_len // 128           # number of 128-row blocks
    JW = HPC * D                # 512: per-core projection width

    nc = bass.Bass()
    x_d = nc.declare_dram_parameter("x", [t_len, M], FP32, isOutput=False)
    wq_d = nc.declare_dram_parameter("wq", [M, JW], FP32, isOutput=False)
    wk_d = nc.declare_dram_parameter("wk", [M, JW], FP32, isOutput=False)
    wv_d = nc.declare_dram_parameter("wv", [M, JW], FP32, isOutput=False)
    wo_d = nc.declare_dram_parameter("wo", [JW, M], FP32, isOutput=False)
    cos_d = nc.declare_dram_parameter("cos_t", [t_len, D // 2], FP32, isOutput=False)
    sin_d = nc.declare_dram_parameter("sin_t", [t_len, D // 2], FP32, isOutput=False)
    out_d = nc.declare_dram_parameter("out", [t_len, M], FP32, isOutput=True)

    NM = M // 128               # m-chunks for contraction

    with SplitDrainTileContext(nc) as tc, ExitStack() as top:
        const_pool = top.enter_context(tc.tile_pool(name="const", bufs=1))
        # Persistent per-head activations (bf16):
        #   qT/kT/oT: [d, head, t];  v: [t-block rows, head*d cols]
        act_pool = top.enter_context(tc.tile_pool(name="acts", bufs=1))
        qT = act_pool.tile([128, HPC, t_len], BF16, tag="qT")
        kT = act_pool.tile([128, HPC, t_len], BF16, tag="kT")
        oT = act_pool.tile([128, HPC, t_len], BF16, tag="oT")
        v_sb = act_pool.tile([128, NT, JW], BF16, tag="v")

        # constants
        ident = const_pool.tile([128, 128], BF16, tag="ident")
        make_identity(nc, ident)
        negmask = const_pool.tile([128, 128], FP32, tag="negmask")
        nc.gpsimd.memset(negmask, 0.0)
        # (x - y) >= 0 keeps 0.0; strictly-upper (j > i) becomes NEG_INF
        nc.gpsimd.affine_select(
            out=negmask, in_=negmask,
            compare_op=mybir.AluOpType.is_ge,
            fill=NEG_INF, base=0,
            pattern=[[-1, 128]], channel_multiplier=1,
        )
        eps_t = const_pool.tile([128, 1], FP32, tag="eps")
        nc.vector.memset(eps_t, NORM_EPS)
        # rope tables, [t-block partition rows, t-block idx, d/2]
        cos_sb = const_pool.tile([128, NT, D // 2], FP32, tag="cos")
        sin_sb = const_pool.tile([128, NT, D // 2], FP32, tag="sin")
        nc.sync.dma_start(out=cos_sb, in_=cos_d.rearrange("(n p) c -> p n c", p=128))
        nc.sync.dma_start(out=sin_sb, in_=sin_d.rearrange("(n p) c -> p n c", p=128))

        # ---------------- Phase 1: projections + norm + rope -----------------
        with ExitStack() as ph1:
            wpool = ph1.enter_context(tc.tile_pool(name="wqkv", bufs=1))
            wq_sb = wpool.tile([128, NM, JW], BF16, tag="wq")
            wk_sb = wpool.tile([128, NM, JW], BF16, tag="wk")
            wv_sb = wpool.tile([128, NM, JW], BF16, tag="wv")
            for m in range(NM):
                nc.gpsimd.dma_start(out=wq_sb[:, m, :], in_=wq_d[m * 128:(m + 1) * 128, :])
                nc.gpsimd.dma_start(out=wk_sb[:, m, :], in_=wk_d[m * 128:(m + 1) * 128, :])
                nc.gpsimd.dma_start(out=wv_sb[:, m, :], in_=wv_d[m * 128:(m + 1) * 128, :])

            xpool = ph1.enter_context(tc.tile_pool(name="xstage", bufs=2))
            qkpool = ph1.enter_context(tc.tile_pool(name="qkstage", bufs=2))
            ppsum = ph1.enter_context(
                tc.tile_pool(name="proj_psum", bufs=2, space=bass.MemorySpace.PSUM))
            tpsum = ph1.enter_context(
                tc.tile_pool(name="qkt_psum", bufs=2, space=bass.MemorySpace.PSUM))

            for ti in range(NT):
                # x block, cast fp32->bf16 during SWDGE DMA
                xbf = xpool.tile([128, M], BF16, tag="xbf")
                nc.gpsimd.dma_start(out=xbf, in_=x_d[ti * 128:(ti + 1) * 128, :])
                # transpose to [m, t] stationary blocks on the PE
                xT = xpool.tile([128, NM, 128], BF16, tag="xT")
                for m in range(NM):
                    xtp = tpsum.tile([128, 128], BF16, tag="pst")
                    nc.tensor.transpose(xtp, xbf[:, m * 128:(m + 1) * 128], ident)
                    cp = (nc.scalar.copy if (m % 2 == 0) else nc.vector.tensor_copy)
                    cp(out=xT[:, m, :], in_=xtp)

                ps_q = ppsum.tile([128, JW], FP32, tag="ps_q")
                ps_k = ppsum.tile([128, JW], FP32, tag="ps_k")
                ps_v = ppsum.tile([128, JW], FP32, tag="ps_v")
                for m in range(NM):
                    st, sp = (m == 0), (m == NM - 1)
                    nc.tensor.matmul(ps_q, xT[:, m, :], wq_sb[:, m, :], start=st, stop=sp)
                    nc.tensor.matmul(ps_k, xT[:, m, :], wk_sb[:, m, :], start=st, stop=sp)
                    nc.tensor.matmul(ps_v, xT[:, m, :], wv_sb[:, m, :], start=st, stop=sp)

                # v: evacuate+cast
                nc.vector.tensor_copy(out=v_sb[:, ti, :], in_=ps_v)

                # q/k: rms-norm + rope + cast + transpose
                for name, ps, dstT in (("q", ps_q, qT), ("k", ps_k, kT)):
                    sq = qkpool.tile([128, JW], FP32, tag="sq")
                    nc.scalar.activation(out=sq, in_=ps,
                                         func=mybir.ActivationFunctionType.Square)
                    rstd = qkpool.tile([128, HPC], FP32, tag="rstd")
                    for h in range(HPC):
                        nc.vector.reduce_sum(out=rstd[:, h:h + 1],
                                             in_=sq[:, h * D:(h + 1) * D],
                                             axis=mybir.AxisListType.X)
                    # rstd = 1/sqrt(sum/D + eps)
                    nc.scalar.activation(out=rstd, in_=rstd,
                                         func=mybir.ActivationFunctionType.Sqrt,
                                         bias=eps_t, scale=1.0 / D)
                    nc.vector.reciprocal(out=rstd, in_=rstd)
                    # normalize (ACT, per-head scale) -> fp32 staging
                    qn = qkpool.tile([128, HPC, D], FP32, tag="qn")
                    for h in range(HPC):
                        nc.scalar.activation(out=qn[:, h, :], in_=ps[:, h * D:(h + 1) * D],
                                             func=mybir.ActivationFunctionType.Copy,
                                             scale=rstd[:, h:h + 1])
                    # rope (batched over heads) -> bf16
                    e = qn[:, :, 0:D // 2]
                    o = qn[:, :, D // 2:D]
                    cos_c = cos_sb[:, ti:ti + 1, :].to_broadcast([128, HPC, D // 2])
                    sin_c = sin_sb[:, ti:ti + 1, :].to_broadcast([128, HPC, D // 2])
                    t1 = qkpool.tile([128, HPC, D // 2], FP32, tag="t1")
                    t2 = qkpool.tile([128, HPC, D // 2], FP32, tag="t2")
                    qb = qkpool.tile([128, HPC, D], BF16, tag="qb")
                    nc.vector.tensor_mul(t1, e, cos_c)
                    nc.vector.tensor_mul(t2, o, sin_c)
                    nc.vector.tensor_sub(qb[:, :, 0:D // 2], t1, t2)
                    nc.vector.tensor_mul(t1, e, sin_c)
                    nc.vector.tensor_mul(t2, o, cos_c)
                    nc.vector.tensor_add(qb[:, :, D // 2:D], t1, t2)
                    # transpose each head block onto [d, t]
                    for h in range(HPC):
                        pst = tpsum.tile([128, 128], BF16, tag="pst")
                        nc.tensor.transpose(pst, qb[:, h, :], ident)
                        cp = (nc.scalar.copy if (h % 2 == 0) else nc.vector.tensor_copy)
                        cp(out=dstT[:, h, ti * 128:(ti + 1) * 128], in_=pst)

        # wo loads can start during attention (pool lives to kernel end; must
        # be opened before phase-2 pools for LIFO release order)
        wopool = top.enter_context(tc.tile_pool(name="wo", bufs=1))
        wo_sb = wopool.tile([128, HPC, M], BF16, tag="wo")
        for h in range(HPC):
            nc.gpsimd.dma_start(out=wo_sb[:, h, :], in_=wo_d[h * D:(h + 1) * D, :])

        # ---------------- Phase 2: causal attention per head -----------------
        with ExitStack() as ph2:
            spool = ph2.enter_context(
                tc.tile_pool(name="s_psum", bufs=4, space=bass.MemorySpace.PSUM))
            opool = ph2.enter_context(
                tc.tile_pool(name="o_psum", bufs=2, space=bass.MemorySpace.PSUM))
            ptpool = ph2.enter_context(
                tc.tile_pool(name="pt_psum", bufs=2, space=bass.MemorySpace.PSUM))
            papool = ph2.enter_context(tc.tile_pool(name="p_sb", bufs=2))
            ptsb = ph2.enter_context(tc.tile_pool(name="pt_sb", bufs=4))
            dpool = ph2.enter_context(tc.tile_pool(name="denom", bufs=3))

            for h in range(HPC):
                for i in range(NT):
                    nj = i + 1                      # causal: j blocks 0..i
                    p_sb = papool.tile([128, t_len], BF16, tag="p")
                    den4 = dpool.tile([128, (NT + 3) // 4], FP32, tag="den4")
                    nchunks = (nj + 3) // 4
                    schunks = []
                    for c in range(nchunks):
                        j0 = c * 4
                        ncols = min(4, nj - j0) * 128
                        ps_s = spool.tile([128, 512], FP32, tag="s")
                        nc.tensor.matmul(ps_s[:, 0:ncols],
                                         qT[:, h, i * 128:(i + 1) * 128],
                                         kT[:, h, j0 * 128:j0 * 128 + ncols])
                        schunks.append((ps_s, j0, ncols))
                    # mask diagonal block (sits in the last chunk)
                    ps_last, j0_last, ncols_last = schunks[-1]
                    dcol = (i - j0_last) * 128
                    nc.vector.tensor_add(out=ps_last[:, dcol:dcol + 128],
                                         in0=ps_last[:, dcol:dcol + 128], in1=negmask)
                    # exp + row-sum per chunk
                    for c, (ps_s, j0, ncols) in enumerate(schunks):
                        nc.scalar.activation(out=p_sb[:, j0 * 128:j0 * 128 + ncols],
                                             in_=ps_s[:, 0:ncols],
                                             func=mybir.ActivationFunctionType.Exp,
                                             scale=QK_SCALE,
                                             accum_out=den4[:, c:c + 1])
                    denom = dpool.tile([128, 1], FP32, tag="denom")
                    if nchunks > 1:
                        nc.vector.reduce_sum(out=denom, in_=den4[:, 0:nchunks],
                                             axis=mybir.AxisListType.X)
                    else:
                        nc.vector.tensor_copy(out=denom, in_=den4[:, 0:1])
                    recip = dpool.tile([128, 1], FP32, tag="recip")
                    nc.vector.reciprocal(out=recip, in_=denom)

                    # o = p @ v  (pT blocks stationary)
                    ps_o = opool.tile([128, 128], FP32, tag="o")
                    for j in range(nj):
                        pst = ptpool.tile([128, 128], BF16, tag="pt")
                        nc.tensor.transpose(pst, p_sb[:, j * 128:(j + 1) * 128], ident)
                        pt_t = ptsb.tile([128, 128], BF16, tag="pts")
                        cp = (nc.scalar.copy if (j % 2 == 0) else nc.vector.tensor_copy)
                        cp(out=pt_t, in_=pst)
                        nc.tensor.matmul(ps_o, pt_t, v_sb[:, j, h * D:(h + 1) * D],
                                         start=(j == 0), stop=(j == nj - 1))
                    # normalize rows while evacuating, then transpose to [d, t]
                    ob = dpool.tile([128, 128], BF16, tag="ob")
                    nc.scalar.activation(out=ob, in_=ps_o,
                                         func=mybir.ActivationFunctionType.Copy,
                                         scale=recip)
                    pso_t = ptpool.tile([128, 128], BF16, tag="pt")
                    nc.tensor.transpose(pso_t, ob, ident)
                    cp = (nc.scalar.copy if (i % 2 == 0) else nc.vector.tensor_copy)
                    cp(out=oT[:, h, i * 128:(i + 1) * 128], in_=pso_t)

        # ---------------- Phase 3: output projection -----------------
        with ExitStack() as ph3:
            upool = ph3.enter_context(
                tc.tile_pool(name="out_psum", bufs=2, space=bass.MemorySpace.PSUM))
            ospool = ph3.enter_context(tc.tile_pool(name="out_sb", bufs=2))
            for ti in range(NT):
                ps_u = upool.tile([128, M], FP32, tag="u")
                for h in range(HPC):
                    for mc in range(M // 512):
                        nc.tensor.matmul(ps_u[:, mc * 512:(mc + 1) * 512],
                                         oT[:, h, ti * 128:(ti + 1) * 128],
                                         wo_sb[:, h, mc * 512:(mc + 1) * 512],
                                         start=(h == 0), stop=(h == HPC - 1))
                o_sb = ospool.tile([128, M], FP32, tag="osb")
                for mc in range(M // 512):
                    cp = (nc.scalar.copy if (mc % 2 == 0) else nc.vector.tensor_copy)
                    cp(out=o_sb[:, mc * 512:(mc + 1) * 512],
                       in_=ps_u[:, mc * 512:(mc + 1) * 512])
                nc.sync.dma_start(out=out_d[ti * 128:(ti + 1) * 128, :], in_=o_sb)

    return nc


def rope_tables(t_len=T):
    pos = np.arange(t_len, dtype=np.float64)[:, None]
    dims = np.arange(D // 2, dtype=np.float64)
    freqs = ROTARY_BASE ** (-dims / (D // 2))[None, :]
    rad = pos * freqs
    return np.cos(rad).astype(np.float32), np.sin(rad).astype(np.float32)


_NC_CACHE = {}


def make_in_maps(x, wq, wk, wv, wo, t_len=T):
    cos_t, sin_t = rope_tables(t_len)
    in_maps = []
    for c in range(N_CORES):
        b, g = divmod(c, N_CORES // B)
        hs = slice(g * HPC, (g + 1) * HPC)
        in_maps.append({
            "x": np.ascontiguousarray(x[b]),
            "wq": np.ascontiguousarray(wq[:, hs, :].reshape(M, HPC * D)),
            "wk": np.ascontiguousarray(wk[:, hs, :].reshape(M, HPC * D)),
            "wv": np.ascontiguousarray(wv[:, hs, :].reshape(M, HPC * D)),
            "wo": np.ascontiguousarray(wo[hs].reshape(HPC * D, M)),
            "cos_t": cos_t,
            "sin_t": sin_t,
        })
    return in_maps


def kernel(x, wq, wk, wv, wo):
    if T not in _NC_CACHE:
        _NC_CACHE[T] = build_nc(T)
    nc = _NC_CACHE[T]
    in_maps = make_in_maps(x, wq, wk, wv, wo)
    res = run_bass_kernel_spmd(nc, in_maps, list(range(N_CORES)))
    gpb = N_CORES // B
    out = np.stack([
        sum(res.results[b * gpb + g]["out"].astype(np.float64) for g in range(gpb))
        for b in range(B)
    ]).astype(np.float32)
    return out


# revision 16
# speedup vs baseline: 85.1668x; 85.1668x over previous
"""Trainium2 Bass kernel for nn_MultiHeadAttention (B=2, T=2048, M=2048, H=16, D=128).

Sharding: 8 cores = batch(2) x head-groups(4).  Each core computes, for its
batch b and 4 heads: q/k/v projections, qk-RMSNorm, RoPE, causal attention,
and its partial contribution o @ wo to the output.  Host sums the 4 head-group
partials per batch.

All matmuls run in bf16 (fp32 accumulation in PSUM); norm/rope/softmax in fp32.
"""
import sys

BASS_PATH = "/opt/trn_rl_repo"
if BASS_PATH not in sys.path:
    sys.path.insert(0, BASS_PATH)

import numpy as np
from contextlib import ExitStack

import concourse.bass as bass
import concourse.tile as tile
from concourse import mybir
from concourse.bass_utils import run_bass_kernel_spmd
from concourse.vector_clock import ScopedClock
from concourse.masks import make_identity

FP32 = mybir.dt.float32
BF16 = mybir.dt.bfloat16

B, T, M, D = 2, 2048, 2048, 128
H = M // D                      # 16 heads total
HPC = 4                         # heads per core
N_CORES = 8
QK_SCALE = 1.0 / D
NORM_EPS = 1e-6
ROTARY_BASE = 10000.0
NEG_INF = -1e30


def _max_waits(inst):
    # The pinned walrus accepts a single sync-wait command per instruction.
    return 1


class SplitDrainTileContext(tile.TileContext):
    """TileContext that splits excess sem waits across nofuse NOPs.

    The pinned walrus rejects more than one sync-wait command on an
    instruction ("Too many sync wait commands"); distribute the excess
    one-per-NOP on the same engine ahead of the instruction.
    """

    def _commit_and_lower(self, inst, original_block, old_bb_map, bb_to_exit_bb):
        si = getattr(inst, "sync_info", None)
        eng = getattr(inst, "engine", None)
        cap = _max_waits(inst)
        if (si is not None and si.on_wait and len(si.on_wait) > cap
                and eng is not None and eng != mybir.EngineType.Unassigned):
            waits = list(si.on_wait)
            excess, keep = waits[:-cap], waits[-cap:]
            inst.sync_info = mybir.SyncInfo(
                on_wait=keep, on_update=list(si.on_update or []))
            for w in excess:
                nop = mybir.InstNoOp(
                    name=self.nc.get_next_instruction_name(),
                    engine=eng,
                    bass_nofuse=True,
                    sync_info=mybir.SyncInfo(on_wait=[w], on_update=[]),
                )
                super()._commit_and_lower(nop, original_block, old_bb_map,
                                          bb_to_exit_bb)
        return super()._commit_and_lower(inst, original_block, old_bb_map,
                                         bb_to_exit_bb)

    def _drain_and_barrier(self, tick_clock, wait_clock):
        probe = self.nc.sync.nop(nofuse=True)
        wait_clock.add_sem_waits(probe.ins, ScopedClock({None: tick_clock.global_clock}))
        si = probe.ins.sync_info
        waits = list(si.on_wait) if si and si.on_wait else []
        updates = list(si.on_update) if si and si.on_update else []
        if len(waits) > 1:
            probe.ins.sync_info = mybir.SyncInfo(on_wait=waits[:1], on_update=updates)
            for w in waits[1:]:
                nop = self.nc.sync.nop(nofuse=True)
                nop.ins.sync_info = mybir.SyncInfo(on_wait=[w], on_update=[])
        self.nc.sync.drain()
        self.nc.all_engine_barrier()
        popped = self.nc._tile_sem_poison_stack.pop()
        assert popped is self._sem_poison
        self.nc.clear_and_free_semaphores(list(self.sems.allocated().values()))
        self.nc.all_engine_barrier()


def build_nc(t_len=T, reps=1):
    """Emit the per-core SPMD program.

    t_len parameterized for small-scale sim; reps>1 wraps the body in a
    hardware loop for on-device timing (the ~100ms axon dispatch overhead
    swamps a single run).
    """
    NT = t_len // 128           # number of 128-row blocks
    JW = HPC * D                # 512: per-core projection width

    nc = bass.Bass()
    x_d = nc.declare_dram_parameter("x", [t_len, M], FP32, isOutput=False)
    wq_d = nc.declare_dram_parameter("wq", [M, JW], FP32, isOutput=False)
    wk_d = nc.declare_dram_parameter("wk", [M, JW], FP32, isOutput=False)
    wv_d = nc.declare_dram_parameter("wv", [M, JW], FP32, isOutput=False)
    wo_d = nc.declare_dram_parameter("wo", [JW, M], FP32, isOutput=False)
    cos_d = nc.declare_dram_parameter("cos_t", [t_len, D // 2], FP32, isOutput=False)
    sin_d = nc.declare_dram_parameter("sin_t", [t_len, D // 2], FP32, isOutput=False)
    out_d = nc.declare_dram_parameter("out", [t_len, M], FP32, isOutput=True)

    NM = M // 128               # m-chunks for contraction

    with SplitDrainTileContext(nc) as tc, ExitStack() as top:
        const_pool = top.enter_context(tc.tile_pool(name="const", bufs=1))
        # Persistent per-head activations (bf16):
        #   qT/kT/oT: [d, head, t];  v: [t-block rows, head*d cols]
        act_pool = top.enter_context(tc.tile_pool(name="acts", bufs=1))
        qT = act_pool.tile([128, HPC, t_len], BF16, tag="qT")
        kT = act_pool.tile([128, HPC, t_len], BF16, tag="kT")
        oT = act_pool.tile([128, HPC, t_len], BF16, tag="oT")
        v_sb = act_pool.tile([128, NT, JW], BF16, tag="v")

        # constants
        ident = const_pool.tile([128, 128], BF16, tag="ident")
        make_identity(nc, ident)
        negmask = const_pool.tile([128, 128], FP32, tag="negmask")
        nc.gpsimd.memset(negmask, 0.0)
        # (x - y) >= 0 keeps 0.0; strictly-upper (j > i) becomes NEG_INF
        nc.gpsimd.affine_select(
            out=negmask, in_=negmask,
            compare_op=mybir.AluOpType.is_ge,
            fill=NEG_INF, base=0,
            pattern=[[-1, 128]], channel_multiplier=1,
        )
        eps_t = const_pool.tile([128, 1], FP32, tag="eps")
        nc.vector.memset(eps_t, NORM_EPS)
        # rope tables, [t-block partition rows, t-block idx, d/2]
        cos_sb = const_pool.tile([128, NT, D // 2], FP32, tag="cos")
        sin_sb = const_pool.tile([128, NT, D // 2], FP32, tag="sin")
        nc.sync.dma_start(out=cos_sb, in_=cos_d.rearrange("(n p) c -> p n c", p=128))
        nc.sync.dma_start(out=sin_sb, in_=sin_d.rearrange("(n p) c -> p n c", p=128))

        # ---------------- Phase 1: projections + norm + rope -----------------
        def _emit_phase1():
            with ExitStack() as ph1:
                wpool = ph1.enter_context(tc.tile_pool(name="wqkv", bufs=1))
                wq_sb = wpool.tile([128, NM, JW], BF16, tag="wq")
                wk_sb = wpool.tile([128, NM, JW], BF16, tag="wk")
                wv_sb = wpool.tile([128, NM, JW], BF16, tag="wv")
                for m in range(NM):
                    nc.gpsimd.dma_start(out=wq_sb[:, m, :], in_=wq_d[m * 128:(m + 1) * 128, :])
                    nc.gpsimd.dma_start(out=wk_sb[:, m, :], in_=wk_d[m * 128:(m + 1) * 128, :])
                    nc.gpsimd.dma_start(out=wv_sb[:, m, :], in_=wv_d[m * 128:(m + 1) * 128, :])

                xpool = ph1.enter_context(tc.tile_pool(name="xstage", bufs=2))
                qkpool = ph1.enter_context(tc.tile_pool(name="qkstage", bufs=2))
                ppsum = ph1.enter_context(
                    tc.tile_pool(name="proj_psum", bufs=2, space=bass.MemorySpace.PSUM))
                tpsum = ph1.enter_context(
                    tc.tile_pool(name="qkt_psum", bufs=2, space=bass.MemorySpace.PSUM))

                for ti in range(NT):
                    # x block, cast fp32->bf16 during SWDGE DMA
                    xbf = xpool.tile([128, M], BF16, tag="xbf")
                    nc.gpsimd.dma_start(out=xbf, in_=x_d[ti * 128:(ti + 1) * 128, :])
                    # transpose to [m, t] stationary blocks on the PE
                    xT = xpool.tile([128, NM, 128], BF16, tag="xT")
                    for m in range(NM):
                        xtp = tpsum.tile([128, 128], BF16, tag="pst")
                        nc.tensor.transpose(xtp, xbf[:, m * 128:(m + 1) * 128], ident)
                        cp = (nc.scalar.copy if (m % 2 == 0) else nc.vector.tensor_copy)
                        cp(out=xT[:, m, :], in_=xtp)

                    ps_q = ppsum.tile([128, JW], FP32, tag="ps_q")
                    ps_k = ppsum.tile([128, JW], FP32, tag="ps_k")
                    ps_v = ppsum.tile([128, JW], FP32, tag="ps_v")
                    for m in range(NM):
                        st, sp = (m == 0), (m == NM - 1)
                        nc.tensor.matmul(ps_q, xT[:, m, :], wq_sb[:, m, :], start=st, stop=sp)
                        nc.tensor.matmul(ps_k, xT[:, m, :], wk_sb[:, m, :], start=st, stop=sp)
                        nc.tensor.matmul(ps_v, xT[:, m, :], wv_sb[:, m, :], start=st, stop=sp)

                    # v: evacuate+cast
                    nc.vector.tensor_copy(out=v_sb[:, ti, :], in_=ps_v)

                    # q/k: rms-norm + rope + cast + transpose
                    for _name, ps, dstT in (("q", ps_q, qT), ("k", ps_k, kT)):
                        sq = qkpool.tile([128, JW], FP32, tag="sq")
                        nc.scalar.activation(out=sq, in_=ps,
                                             func=mybir.ActivationFunctionType.Square)
                        rstd = qkpool.tile([128, HPC], FP32, tag="rstd")
                        for h in range(HPC):
                            nc.vector.reduce_sum(out=rstd[:, h:h + 1],
                                                 in_=sq[:, h * D:(h + 1) * D],
                                                 axis=mybir.AxisListType.X)
                        # rstd = 1/sqrt(sum/D + eps)
                        nc.scalar.activation(out=rstd, in_=rstd,
                                             func=mybir.ActivationFunctionType.Sqrt,
                                             bias=eps_t, scale=1.0 / D)
                        nc.vector.reciprocal(out=rstd, in_=rstd)
                        # normalize (ACT, per-head scale) -> fp32 staging
                        qn = qkpool.tile([128, HPC, D], FP32, tag="qn")
                        for h in range(HPC):
                            nc.scalar.activation(out=qn[:, h, :], in_=ps[:, h * D:(h + 1) * D],
                                                 func=mybir.ActivationFunctionType.Copy,
                                                 scale=rstd[:, h:h + 1])
                        # rope (batched over heads) -> bf16
                        e = qn[:, :, 0:D // 2]
                        o = qn[:, :, D // 2:D]
                        cos_c = cos_sb[:, ti:ti + 1, :].to_broadcast([128, HPC, D // 2])
                        sin_c = sin_sb[:, ti:ti + 1, :].to_broadcast([128, HPC, D // 2])
                        t1 = qkpool.tile([128, HPC, D // 2], FP32, tag="t1")
                        t2 = qkpool.tile([128, HPC, D // 2], FP32, tag="t2")
                        qb = qkpool.tile([128, HPC, D], BF16, tag="qb")
                        nc.vector.tensor_mul(t1, e, cos_c)
                        nc.vector.tensor_mul(t2, o, sin_c)
                        nc.vector.tensor_sub(qb[:, :, 0:D // 2], t1, t2)
                        nc.vector.tensor_mul(t1, e, sin_c)
                        nc.vector.tensor_mul(t2, o, cos_c)
                        nc.vector.tensor_add(qb[:, :, D // 2:D], t1, t2)
                        # transpose each head block onto [d, t]
                        for h in range(HPC):
                            pst = tpsum.tile([128, 128], BF16, tag="pst")
                            nc.tensor.transpose(pst, qb[:, h, :], ident)
                            cp = (nc.scalar.copy if (h % 2 == 0) else nc.vector.tensor_copy)
                            cp(out=dstT[:, h, ti * 128:(ti + 1) * 128], in_=pst)

        # ------- Phases 2+3: attention per head, then output projection ------
        def _emit_phase23(rep_stack):
            # wo loads can start during attention (pool outlives phases 2+3;
            # opened before phase-2 pools for LIFO release order)
            wopool = rep_stack.enter_context(tc.tile_pool(name="wo", bufs=1))
            wo_sb = wopool.tile([128, HPC, M], BF16, tag="wo")
            for h in range(HPC):
                nc.gpsimd.dma_start(out=wo_sb[:, h, :], in_=wo_d[h * D:(h + 1) * D, :])

            with ExitStack() as ph2:
                spool = ph2.enter_context(
                    tc.tile_pool(name="s_psum", bufs=4, space=bass.MemorySpace.PSUM))
                opool = ph2.enter_context(
                    tc.tile_pool(name="o_psum", bufs=2, space=bass.MemorySpace.PSUM))
                ptpool = ph2.enter_context(
                    tc.tile_pool(name="pt_psum", bufs=2, space=bass.MemorySpace.PSUM))
                papool = ph2.enter_context(tc.tile_pool(name="p_sb", bufs=2))
                ptsb = ph2.enter_context(tc.tile_pool(name="pt_sb", bufs=4))
                dpool = ph2.enter_context(tc.tile_pool(name="denom", bufs=3))

                for h in range(HPC):
                    for i in range(NT):
                        nj = i + 1                      # causal: j blocks 0..i
                        p_sb = papool.tile([128, t_len], BF16, tag="p")
                        den4 = dpool.tile([128, (NT + 3) // 4], FP32, tag="den4")
                        nchunks = (nj + 3) // 4
                        schunks = []
                        for c in range(nchunks):
                            j0 = c * 4
                            ncols = min(4, nj - j0) * 128
                            ps_s = spool.tile([128, 512], FP32, tag="s")
                            nc.tensor.matmul(ps_s[:, 0:ncols],
                                             qT[:, h, i * 128:(i + 1) * 128],
                                             kT[:, h, j0 * 128:j0 * 128 + ncols])
                            schunks.append((ps_s, j0, ncols))
                        # mask diagonal block (sits in the last chunk)
                        ps_last, j0_last, _ncols_last = schunks[-1]
                        dcol = (i - j0_last) * 128
                        nc.vector.tensor_add(out=ps_last[:, dcol:dcol + 128],
                                             in0=ps_last[:, dcol:dcol + 128], in1=negmask)
                        # exp + row-sum per chunk
                        for c, (ps_s, j0, ncols) in enumerate(schunks):
                            nc.scalar.activation(out=p_sb[:, j0 * 128:j0 * 128 + ncols],
                                                 in_=ps_s[:, 0:ncols],
                                                 func=mybir.ActivationFunctionType.Exp,
                                                 scale=QK_SCALE,
                                                 accum_out=den4[:, c:c + 1])
                        denom = dpool.tile([128, 1], FP32, tag="denom")
                        if nchunks > 1:
                            nc.vector.reduce_sum(out=denom, in_=den4[:, 0:nchunks],
                                                 axis=mybir.AxisListType.X)
                        else:
                            nc.vector.tensor_copy(out=denom, in_=den4[:, 0:1])
                        recip = dpool.tile([128, 1], FP32, tag="recip")
                        nc.vector.reciprocal(out=recip, in_=denom)

                        # o = p @ v  (pT blocks stationary)
                        ps_o = opool.tile([128, 128], FP32, tag="o")
                        for j in range(nj):
                            pst = ptpool.tile([128, 128], BF16, tag="pt")
                            nc.tensor.transpose(pst, p_sb[:, j * 128:(j + 1) * 128], ident)
                            pt_t = ptsb.tile([128, 128], BF16, tag="pts")
                            cp = (nc.scalar.copy if (j % 2 == 0) else nc.vector.tensor_copy)
                            cp(out=pt_t, in_=pst)
                            nc.tensor.matmul(ps_o, pt_t, v_sb[:, j, h * D:(h + 1) * D],
                                             start=(j == 0), stop=(j == nj - 1))
                        # normalize rows while evacuating, then transpose to [d, t]
                        ob = dpool.tile([128, 128], BF16, tag="ob")
                        nc.scalar.activation(out=ob, in_=ps_o,
                                             func=mybir.ActivationFunctionType.Copy,
                                             scale=recip)
                        pso_t = ptpool.tile([128, 128], BF16, tag="pt")
                        nc.tensor.transpose(pso_t, ob, ident)
                        cp = (nc.scalar.copy if (i % 2 == 0) else nc.vector.tensor_copy)
                        cp(out=oT[:, h, i * 128:(i + 1) * 128], in_=pso_t)

            with ExitStack() as ph3:
                upool = ph3.enter_context(
                    tc.tile_pool(name="out_psum", bufs=2, space=bass.MemorySpace.PSUM))
                ospool = ph3.enter_context(tc.tile_pool(name="out_sb", bufs=2))
                for ti in range(NT):
                    ps_u = upool.tile([128, M], FP32, tag="u")
                    for h in range(HPC):
                        for mc in range(M // 512):
                            nc.tensor.matmul(ps_u[:, mc * 512:(mc + 1) * 512],
                                             oT[:, h, ti * 128:(ti + 1) * 128],
                                             wo_sb[:, h, mc * 512:(mc + 1) * 512],
                                             start=(h == 0), stop=(h == HPC - 1))
                    o_sb = ospool.tile([128, M], FP32, tag="osb")
                    for mc in range(M // 512):
                        cp = (nc.scalar.copy if (mc % 2 == 0) else nc.vector.tensor_copy)
                        cp(out=o_sb[:, mc * 512:(mc + 1) * 512],
                           in_=ps_u[:, mc * 512:(mc + 1) * 512])
                    nc.sync.dma_start(out=out_d[ti * 128:(ti + 1) * 128, :], in_=o_sb)

        def _emit_body():
            _emit_phase1()
            with ExitStack() as rep_stack:
                _emit_phase23(rep_stack)

        # For_i trips an "ISA wrong length" bug in the pinned walrus, so
        # timing reps are python-unrolled.
        for _ in range(reps):
            _emit_body()

    return nc


def rope_tables(t_len=T):
    pos = np.arange(t_len, dtype=np.float64)[:, None]
    dims = np.arange(D // 2, dtype=np.float64)
    freqs = ROTARY_BASE ** (-dims / (D // 2))[None, :]
    rad = pos * freqs
    return np.cos(rad).astype(np.float32), np.sin(rad).astype(np.float32)


_NC_CACHE = {}


def make_in_maps(x, wq, wk, wv, wo, t_len=T):
    cos_t, sin_t = rope_tables(t_len)
    in_maps = []
    for c in range(N_CORES):
        b, g = divmod(c, N_CORES // B)
        hs = slice(g * HPC, (g + 1) * HPC)
        in_maps.append({
            "x": np.ascontiguousarray(x[b]),
            "wq": np.ascontiguousarray(wq[:, hs, :].reshape(M, HPC * D)),
            "wk": np.ascontiguousarray(wk[:, hs, :].reshape(M, HPC * D)),
            "wv": np.ascontiguousarray(wv[:, hs, :].reshape(M, HPC * D)),
            "wo": np.ascontiguousarray(wo[hs].reshape(HPC * D, M)),
            "cos_t": cos_t,
            "sin_t": sin_t,
        })
    return in_maps


def kernel(x, wq, wk, wv, wo):
    if T not in _NC_CACHE:
        _NC_CACHE[T] = build_nc(T)
    nc = _NC_CACHE[T]
    in_maps = make_in_maps(x, wq, wk, wv, wo)
    res = run_bass_kernel_spmd(nc, in_maps, list(range(N_CORES)))
    gpb = N_CORES // B
    out = np.stack([
        sum(res.results[b * gpb + g]["out"].astype(np.float64) for g in range(gpb))
        for b in range(B)
    ]).astype(np.float32)
    return out
